# revision 1
# baseline (speedup 1.0000x reference)
"""Trainium2 Bass kernel for nn_Pndb_43344809951805 (scatter_memory).

Data-parallel over batch B=16 across 8 NeuronCores (2 batches/core).
Phase 1 writes the [Q,D] memory A (mean over B -> AllReduce), phase 2 reads it.
All big matmuls run in bf16 (full PE rate); residual path stays f32.
"""
import sys

sys.path.insert(0, "/opt/trn_rl_repo")

import numpy as np
import ml_dtypes

import concourse.bass as bass
import concourse.bacc as bacc
import concourse.mybir as mybir
import concourse.tile as tile
from concourse import masks
from concourse.bass_utils import run_bass_kernel_spmd

F32 = mybir.dt.float32
BF16 = mybir.dt.bfloat16
AF = mybir.ActivationFunctionType
ALU = mybir.AluOpType
BF = ml_dtypes.bfloat16

B, S, D, Q = 16, 2048, 1024, 64
NCORES = 8
BL = B // NCORES          # local batches per core
SBLK = 512                # s-block (matmul moving free dim)
NSB = S // SBLK           # 4 s-blocks per batch
NCH = S // 128            # 16 s-chunks per batch
NJ = D // 128             # 8 contraction chunks
NI = D // 128             # 8 output-dim chunks
CPB = SBLK // 128         # 4 chunks per s-block

_prog_cache = {}


def _build(bi_v: float, cgate_v: float, stage: str = "full"):
    nc = bacc.Bacc("TRN2", target_bir_lowering=False, debug=False,
                   enable_asserts=False, num_devices=NCORES)

    rawT_d = nc.dram_tensor("rawT", [BL, D, S], BF16, kind="ExternalInput")
    rawN_d = nc.dram_tensor("rawN", [BL, S, D], BF16, kind="ExternalInput")
    pdT_d = nc.dram_tensor("pdT", [BL, D, S], BF16, kind="ExternalInput")
    pdN_d = nc.dram_tensor("pdN", [BL, S, D], F32, kind="ExternalInput")
    wkT_d = nc.dram_tensor("wkT", [D, D], BF16, kind="ExternalInput")
    woT_d = nc.dram_tensor("woT", [D, D], BF16, kind="ExternalInput")
    qT1_d = nc.dram_tensor("qT1", [D, Q], BF16, kind="ExternalInput")
    qT2_d = nc.dram_tensor("qT2", [D, Q], BF16, kind="ExternalInput")
    bkT_d = nc.dram_tensor("bkT", [D, 1], F32, kind="ExternalInput")
    boT_d = nc.dram_tensor("boT", [D, 1], F32, kind="ExternalInput")
    wiB_d = nc.dram_tensor("wiB", [128, D], BF16, kind="ExternalInput")
    wu1B_d = nc.dram_tensor("wu1B", [128, D], F32, kind="ExternalInput")
    wu2B_d = nc.dram_tensor("wu2B", [Q, D], F32, kind="ExternalInput")
    out_d = nc.dram_tensor("out", [BL, S, D], F32, kind="ExternalOutput")

    with tile.TileContext(nc) as tc:
        with (
            tc.tile_pool(name="const", bufs=1) as cp,
            tc.tile_pool(name="dram", bufs=1, space="DRAM") as dram,
        ):
            # ---- constants; only wkT DMA'd up-front (first-MM critical) ----
            ident = cp.tile([128, 128], BF16, tag="ident")
            masks.make_identity(nc, ident[:])
            nbi = cp.tile([128, 1], F32, tag="nbi")
            nc.vector.memset(nbi[:], -bi_v)
            ncg = cp.tile([128, 1], F32, tag="ncg")
            nc.vector.memset(ncg[:], -cgate_v)

            wkT = [cp.tile([128, D], BF16, tag=f"wkT{j}", name=f"wkT{j}")
                   for j in range(NJ)]
            woT = [cp.tile([128, D], BF16, tag=f"woT{j}", name=f"woT{j}")
                   for j in range(NJ)]
            qT1 = [cp.tile([128, Q], BF16, tag=f"qT1{j}", name=f"qT1{j}")
                   for j in range(NJ)]
            qT2 = [cp.tile([128, Q], BF16, tag=f"qT2{j}", name=f"qT2{j}")
                   for j in range(NJ)]
            bkT = [cp.tile([128, 1], F32, tag=f"bkT{j}", name=f"bkT{j}")
                   for j in range(NJ)]
            boT = [cp.tile([128, 1], F32, tag=f"boT{j}", name=f"boT{j}")
                   for j in range(NJ)]
            wiB = cp.tile([128, D], BF16, tag="wiB")
            wu1B = cp.tile([128, D], F32, tag="wu1B")
            wu2B = cp.tile([Q, D], F32, tag="wu2B")
            for j in range(NJ):
                nc.sync.dma_start(wkT[j][:],
                                  wkT_d[j * 128:(j + 1) * 128, :])

            A_acc = cp.tile([Q, D], F32, tag="A_acc")
            A_f32 = cp.tile([Q, D], F32, tag="A_f32")
            A_bf = cp.tile([Q, D], BF16, tag="A_bf")
            awB = cp.tile([128, Q], BF16, tag="awB")
            scrA = cp.tile([Q, D], F32, tag="scrA")
            aw = cp.tile([Q, 1], F32, tag="aw")
            ar_in = dram.tile([Q + 1, D + 8], BF16)
            ar_out = dram.tile([Q + 1, D + 8], BF16)
            awz = cp.tile([Q, 8], BF16, tag="awz")
            nc.vector.memset(awz[:], 0.0)
            zrow = cp.tile([1, D + 8], BF16, tag="zrow")
            nc.vector.memset(zrow[:], 0.0)
            nc.gpsimd.dma_start(ar_in[0:Q, D:D + 8], awz[:])
            nc.gpsimd.dma_start(ar_in[Q:Q + 1, Q:D + 8], zrow[:, Q:D + 8])
            # phase-2 first-block data, prefetched during phase 1
            pdt0 = [cp.tile([128, SBLK], BF16, tag=f"pdt0_{j}",
                            name=f"pdt0_{j}") for j in range(NJ)]
            pdn0 = [cp.tile([128, D], F32, tag=f"pdn0_{c}",
                            name=f"pdn0_{c}") for c in range(CPB)]


            # ================= PHASE 1 =================
            with (
                tc.tile_pool(name="p1", bufs=1) as p1,
                tc.tile_pool(name="p1ps", bufs=1, space="PSUM") as p1ps,
            ):
                def load_rawt(b, sb):
                    ts = []
                    for j in range(NJ):
                        t = p1.tile([128, SBLK], BF16, tag=f"rawt{j}",
                                    name=f"rawt{j}", bufs=2)
                        nc.sync.dma_start(
                            t[:], rawT_d[b, j * 128:(j + 1) * 128,
                                         sb * SBLK:(sb + 1) * SBLK])
                        ts.append(t)
                    return ts

                for b in range(BL):
                    rawt = load_rawt(b, 0)
                    if b == 0:
                        nc.sync.dma_start(wiB[:], wiB_d[:])
                        for j in range(NJ):
                            sl = slice(j * 128, (j + 1) * 128)
                            nc.sync.dma_start(bkT[j][:], bkT_d[sl, :])
                            nc.sync.dma_start(qT1[j][:], qT1_d[sl, :])
                    U = p1.tile([Q, S], BF16, tag="U", bufs=2)
                    Zp = p1.tile([Q, NSB], F32, tag="Zp", bufs=2)
                    A_ps = p1ps.tile([Q, D], F32, tag="A_ps", bufs=1)

                    for sb in range(NSB):
                        # v-gate pre-pass + v for this s-block's chunks
                        Gg = p1.tile([128, CPB], F32, tag="Gg", bufs=2)
                        rns = []
                        for cc in range(CPB):
                            c = sb * CPB + cc
                            rn = p1.tile([128, D], BF16, tag=f"rawn{cc}",
                                         name=f"rawn{cc}", bufs=2)
                            nc.sync.dma_start(
                                rn[:], rawN_d[b, c * 128:(c + 1) * 128, :])
                            rns.append(rn)
                            scr = p1.tile([128, D], BF16, tag="scrb", bufs=2)
                            nc.vector.scalar_tensor_tensor(
                                scr[:], rn[:], 1.0, wiB[:],
                                ALU.mult, ALU.mult,
                                accum_out=Gg[:, cc:cc + 1])
                        nc.scalar.activation(Gg[:], Gg[:], AF.Exp,
                                             scale=-1.0, bias=nbi[:])
                        nc.vector.tensor_scalar_add(Gg[:], Gg[:], 1.0)
                        nc.vector.reciprocal(Gg[:], Gg[:])
                        vs = []
                        for cc in range(CPB):
                            v = p1.tile([128, D], BF16, tag=f"v{cc}",
                                        name=f"v{cc}", bufs=2)
                            nc.vector.tensor_scalar_mul(
                                v[:], rns[cc][:], Gg[:, cc:cc + 1])
                            vs.append(v)

                        nxt = load_rawt(b, sb + 1) if sb + 1 < NSB else None
                        if b == 0 and sb == 0:
                            # phase-2 weights: off the critical DMA path
                            for j in range(NJ):
                                sl = slice(j * 128, (j + 1) * 128)
                                nc.sync.dma_start(woT[j][:], woT_d[sl, :])
                                nc.sync.dma_start(qT2[j][:], qT2_d[sl, :])
                                nc.sync.dma_start(boT[j][:], boT_d[sl, :])
                            nc.sync.dma_start(wu1B[:], wu1B_d[:])
                            nc.sync.dma_start(wu2B[:], wu2B_d[:])
                        if b == 1 and sb == 0:
                            # prefetch phase-2 (b0, sb0) data
                            for j in range(NJ):
                                nc.sync.dma_start(
                                    pdt0[j][:],
                                    pdT_d[0, j * 128:(j + 1) * 128, 0:SBLK])
                            for c in range(CPB):
                                nc.sync.dma_start(
                                    pdn0[c][:],
                                    pdN_d[0, c * 128:(c + 1) * 128, :])

                        # kT matmuls + scores (software-pipelined by one i)
                        sc_ps = p1ps.tile([Q, SBLK], F32, tag="sc_ps", bufs=2)
                        kts = []
                        for i in range(NI):
                            isl = slice(i * 128, (i + 1) * 128)
                            k_ps = p1ps.tile([128, SBLK], F32, tag="k_ps",
                                             bufs=2)
                            for j in range(NJ):
                                nc.tensor.matmul(
                                    k_ps[:], wkT[j][:, isl], rawt[j][:],
                                    start=(j == 0), stop=(j == NJ - 1))
                            kt = p1.tile([128, SBLK], BF16, tag="kt", bufs=3)
                            nc.scalar.activation(kt[:], k_ps[:], AF.Identity,
                                                 bias=bkT[i][:])
                            kts.append(kt)
                            if i >= 1:
                                nc.tensor.matmul(
                                    sc_ps[:], qT1[i - 1][:], kts[i - 1][:],
                                    start=(i - 1 == 0), stop=False,
                                    skip_group_check=True)
                        nc.tensor.matmul(
                            sc_ps[:], qT1[NI - 1][:], kts[NI - 1][:],
                            start=False, stop=True, skip_group_check=True)

                        ssl = slice(sb * SBLK, (sb + 1) * SBLK)
                        nc.scalar.activation(U[:, ssl], sc_ps[:], AF.Exp,
                                             accum_out=Zp[:, sb:sb + 1])
                        # transposes first (decoupled from A matmuls)
                        uts = []
                        for cc in range(CPB):
                            c = sb * CPB + cc
                            ut_ps = p1ps.tile([128, Q], BF16, tag="ut_ps",
                                              bufs=2)
                            nc.tensor.transpose(
                                ut_ps[:], U[:, c * 128:(c + 1) * 128],
                                ident[:Q, :Q])
                            ut = p1.tile([128, Q], BF16, tag="ut", bufs=6)
                            nc.scalar.copy(ut[:], ut_ps[:])
                            uts.append(ut)
                        for cc in range(CPB):
                            c = sb * CPB + cc
                            for h in range(2):
                                hsl = slice(h * 512, (h + 1) * 512)
                                nc.tensor.matmul(
                                    A_ps[:, hsl], uts[cc][:], vs[cc][:, hsl],
                                    start=(c == 0), stop=(c == NCH - 1),
                                    skip_group_check=True)
                        rawt = nxt

                    # A_acc += A_ps / (16 * Z)
                    Z1 = p1.tile([Q, 1], F32, tag="Z1", bufs=2)
                    nc.vector.tensor_reduce(Z1[:], Zp[:], mybir.AxisListType.X,
                                            ALU.add)
                    sA = p1.tile([Q, 1], F32, tag="sA", bufs=2)
                    nc.vector.reciprocal(sA[:], Z1[:])
                    nc.vector.tensor_scalar_mul(sA[:], sA[:], 1.0 / B)
                    if b == 0:
                        nc.vector.tensor_scalar_mul(A_acc[:], A_ps[:], sA[:])
                    else:
                        nc.vector.scalar_tensor_tensor(
                            A_acc[:], A_ps[:], sA[:], A_acc[:],
                            ALU.mult, ALU.add)

                # aw_local = A_acc . Wu2 rides along in the AllReduce
                nc.vector.scalar_tensor_tensor(
                    scrA[:], A_acc[:], 1.0, wu2B[:],
                    ALU.mult, ALU.mult, accum_out=aw[:])
                nc.gpsimd.dma_start(ar_in[0:Q, 0:D], A_acc[:])
                nc.gpsimd.dma_start(
                    ar_in[Q:Q + 1, 0:Q].rearrange("a b -> b a"), aw[:])

            # ---- AllReduce of partial A across the 8 cores ----
            if stage == "p1":
                nc.sync.dma_start(out_d[0, 0:Q, :], A_acc[:])
            else:
                if stage == "p2":
                    arr = ar_in
                else:
                    nc.gpsimd.collective_compute(
                        "AllReduce", ALU.add,
                        replica_groups=[list(range(NCORES))],
                        ins=[ar_in.opt()], outs=[ar_out.opt()],
                    )
                    arr = ar_out
                if stage == "ar":
                    nc.gpsimd.dma_start(A_f32[:], arr[0:Q, 0:D])
                nc.gpsimd.dma_start(A_bf[:], arr[0:Q, 0:D])
                nc.gpsimd.dma_start(
                    awB[:], arr[Q:Q + 1, 0:Q].broadcast_to([128, Q]))

            # ================= PHASE 2 =================
            if stage == "p1":
                pass  # skip phase 2
            else:
              with (
                  tc.tile_pool(name="p2", bufs=1) as p2,
                  tc.tile_pool(name="p2ps", bufs=1, space="PSUM") as p2ps,
              ):
                  def load_pdt(b, sb):
                      ts = []
                      for j in range(NJ):
                          t = p2.tile([128, SBLK], BF16, tag=f"pdt{j}",
                                      name=f"pdt{j}", bufs=2)
                          nc.sync.dma_start(
                              t[:], pdT_d[b, j * 128:(j + 1) * 128,
                                          sb * SBLK:(sb + 1) * SBLK])
                          ts.append(t)
                      return ts

                  def emit_ko(pdt):
                      kot = []
                      for i in range(NI):
                          isl = slice(i * 128, (i + 1) * 128)
                          ko_ps = p2ps.tile([128, SBLK], F32, tag="ko_ps",
                                            bufs=2)
                          for j in range(NJ):
                              nc.tensor.matmul(
                                  ko_ps[:], woT[j][:, isl], pdt[j][:],
                                  start=(j == 0), stop=(j == NJ - 1))
                          kt = p2.tile([128, SBLK], BF16, tag="kot",
                                       name="kot", bufs=16)
                          nc.scalar.activation(kt[:], ko_ps[:], AF.Identity,
                                               bias=boT[i][:])
                          kot.append(kt)
                      return kot

                  def emit_partB(st):
                      (Z2, u2s, b, sb, idx, G1, pdn) = st
                      G2 = p2.tile([128, CPB], F32, tag="G2", bufs=2)
                      SC = p2.tile([128, CPB], F32, tag="SC", bufs=2)
                      if G1 is None:
                          G1 = p2.tile([128, CPB], F32, tag="G1", bufs=2)
                          pdn = []
                          for cc in range(CPB):
                              c = sb * CPB + cc
                              if idx == 0:
                                  pn = pdn0[cc]
                              else:
                                  pn = p2.tile([128, D], F32,
                                               tag=f"pdn{cc}",
                                               name=f"pdn{cc}", bufs=2)
                                  nc.sync.dma_start(
                                      pn[:],
                                      pdN_d[b, c * 128:(c + 1) * 128, :])
                              pdn.append(pn)
                              scr = p2.tile([128, D], F32, tag="scrf",
                                            bufs=2)
                              nc.vector.scalar_tensor_tensor(
                                  scr[:], pn[:], 1.0, wu1B[:],
                                  ALU.mult, ALU.mult,
                                  accum_out=G1[:, cc:cc + 1])
                      for cc in range(CPB):
                          scr2 = p2.tile([128, Q], BF16, tag="scr2",
                                         bufs=2)
                          nc.vector.scalar_tensor_tensor(
                              scr2[:], u2s[cc][:], 1.0, awB[:],
                              ALU.mult, ALU.mult,
                              accum_out=G2[:, cc:cc + 1])
                      # gates: sc = sigmoid(g1 + g2/Z + cg) / Z
                      rz = p2.tile([128, CPB], F32, tag="rz", bufs=2)
                      nc.vector.reciprocal(rz[:], Z2[:])
                      t4 = p2.tile([128, CPB], F32, tag="t4", bufs=2)
                      nc.vector.tensor_mul(t4[:], G2[:], rz[:])
                      nc.vector.tensor_add(t4[:], t4[:], G1[:])
                      e4 = p2.tile([128, CPB], F32, tag="e4", bufs=2)
                      nc.scalar.activation(e4[:], t4[:], AF.Exp,
                                           scale=-1.0, bias=ncg[:])
                      nc.vector.tensor_scalar_add(e4[:], e4[:], 1.0)
                      nc.vector.reciprocal(e4[:], e4[:])
                      nc.vector.tensor_mul(SC[:], e4[:], rz[:])
                      # transposes, then A2 matmuls + fused residual
                      ut2s = []
                      for cc in range(CPB):
                          ut2_ps = p2ps.tile([Q, 128], BF16, tag="ut2_ps",
                                             bufs=2)
                          nc.tensor.transpose(ut2_ps[:], u2s[cc][:],
                                              ident[:, :])
                          ut2 = p2.tile([Q, 128], BF16, tag="ut2", bufs=6)
                          nc.scalar.copy(ut2[:], ut2_ps[:])
                          ut2s.append(ut2)
                      for cc in range(CPB):
                          c = sb * CPB + cc
                          outt = p2.tile([128, D], F32, tag="outt", bufs=3)
                          for h in range(2):
                              hsl = slice(h * 512, (h + 1) * 512)
                              a2_ps = p2ps.tile([128, 512], F32,
                                                tag="a2_ps", bufs=2)
                              nc.tensor.matmul(a2_ps[:], ut2s[cc][:],
                                               A_bf[:, hsl],
                                               start=True, stop=True)
                              nc.vector.scalar_tensor_tensor(
                                  outt[:, hsl], a2_ps[:],
                                  SC[:, cc:cc + 1],
                                  pdn[cc][:, hsl], ALU.mult, ALU.add)
                          nc.sync.dma_start(
                              out_d[b, c * 128:(c + 1) * 128, :], outt[:])

                  all_sb = [(b, sb) for b in range(BL) for sb in range(NSB)]
                  pdt_cur = pdt0
                  pending = []
                  for idx, (b, sb) in enumerate(all_sb):
                      kot = emit_ko(pdt_cur)
                      pdt_nxt = (load_pdt(*all_sb[idx + 1])
                                 if idx + 1 < len(all_sb) else None)
                      # part A: s2 + exp per chunk (A-independent)
                      Z2 = p2.tile([128, CPB], F32, tag="Z2", bufs=4)
                      u2s = []
                      for cc in range(CPB):
                          c = sb * CPB + cc
                          s2_ps = p2ps.tile([128, Q], F32, tag="s2_ps",
                                            bufs=2)
                          for i in range(NI):
                              nc.tensor.matmul(
                                  s2_ps[:],
                                  kot[i][:, cc * 128:(cc + 1) * 128],
                                  qT2[i][:],
                                  start=(i == 0), stop=(i == NI - 1),
                                  skip_group_check=True)
                          u2 = p2.tile([128, Q], BF16, tag=f"u2{cc}",
                                       name=f"u2{cc}", bufs=4)
                          nc.scalar.activation(u2[:], s2_ps[:], AF.Exp,
                                               accum_out=Z2[:, cc:cc + 1])
                          u2s.append(u2)

                      G1e = None
                      pdne = None
                      if idx >= len(all_sb) - 2:
                          G1e = p2.tile([128, CPB], F32, tag="G1L", bufs=2)
                          pdne = []
                          for cc in range(CPB):
                              c = sb * CPB + cc
                              pn = p2.tile([128, D], F32, tag=f"pdnL{cc}",
                                           name=f"pdnL{cc}", bufs=2)
                              nc.sync.dma_start(
                                  pn[:],
                                  pdN_d[b, c * 128:(c + 1) * 128, :])
                              pdne.append(pn)
                              scr = p2.tile([128, D], F32, tag="scrf",
                                            bufs=2)
                              nc.vector.scalar_tensor_tensor(
                                  scr[:], pn[:], 1.0, wu1B[:],
                                  ALU.mult, ALU.mult,
                                  accum_out=G1e[:, cc:cc + 1])

                      depth = 3 if idx <= 4 else 2
                      while len(pending) >= depth:
                          emit_partB(pending.pop(0))
                      pending.append((Z2, u2s, b, sb, idx, G1e, pdne))
                      pdt_cur = pdt_nxt
                  for st in pending:
                      emit_partB(st)

            if stage == "ar":
                nc.sync.dma_start(out_d[0, 0:Q, :], A_f32[:])
                nc.gpsimd.dma_start(out_d[0, 128:256, 0:Q], awB[:])
    nc.compile()
    return nc


def _get_prog(bi_v, cgate_v):
    key = (round(bi_v, 9), round(cgate_v, 9))
    if key not in _prog_cache:
        _prog_cache[key] = _build(bi_v, cgate_v)
    return _prog_cache[key]


def kernel(raw, post_dec, mask, questions, Wk, bk, Wi, bi, Wo, bo,
           Wu1, bu1, Wu2, bu2, b1, _trace=False):
    raw = np.asarray(raw, dtype=np.float32)
    post_dec = np.asarray(post_dec, dtype=np.float32)
    questions = np.asarray(questions, dtype=np.float32)
    Wk = np.asarray(Wk, dtype=np.float32)
    Wo = np.asarray(Wo, dtype=np.float32)

    bi_v = float(np.asarray(bi).reshape(-1)[0])
    cgate_v = float(np.asarray(bu1).reshape(-1)[0]
                    + np.asarray(bu2).reshape(-1)[0]
                    + np.asarray(b1).reshape(-1)[0])
    nc = _get_prog(bi_v, cgate_v)

    inv_sqrt_d = np.float32(1.0 / np.sqrt(D))
    inv_sqrt_q = np.float32(1.0 / np.sqrt(Q))
    wkT = np.ascontiguousarray(Wk.T).astype(BF)
    woT = np.ascontiguousarray(Wo.T).astype(BF)
    qT1 = np.ascontiguousarray(questions.T * inv_sqrt_d).astype(BF)
    qT2 = np.ascontiguousarray(questions.T * inv_sqrt_q).astype(BF)
    bkT = np.ascontiguousarray(np.asarray(bk, np.float32).reshape(D, 1))
    boT = np.ascontiguousarray(np.asarray(bo, np.float32).reshape(D, 1))
    wiB = np.ascontiguousarray(
        np.broadcast_to(np.asarray(Wi, np.float32).reshape(1, D), (128, D))
    ).astype(BF)
    wu1B = np.ascontiguousarray(
        np.broadcast_to(np.asarray(Wu1, np.float32).reshape(1, D), (128, D)))
    wu2B = np.ascontiguousarray(
        np.broadcast_to(np.asarray(Wu2, np.float32).reshape(1, D), (Q, D)))

    in_maps = []
    for r in range(NCORES):
        bs = slice(r * BL, (r + 1) * BL)
        rawT = np.ascontiguousarray(
            raw[bs].transpose(0, 2, 1)).astype(BF)
        rawN = np.ascontiguousarray(raw[bs]).astype(BF)
        pdT = np.ascontiguousarray(
            post_dec[bs].transpose(0, 2, 1)).astype(BF)
        pdN = np.ascontiguousarray(post_dec[bs])
        in_maps.append({
            "rawT": rawT, "rawN": rawN, "pdT": pdT, "pdN": pdN,
            "wkT": wkT, "woT": woT, "qT1": qT1, "qT2": qT2,
            "bkT": bkT, "boT": boT, "wiB": wiB, "wu1B": wu1B, "wu2B": wu2B,
        })

    res = run_bass_kernel_spmd(nc, in_maps, core_ids=list(range(NCORES)),
                               trace=_trace)
    out = np.concatenate([res.results[r]["out"] for r in range(NCORES)],
                         axis=0)
    if _trace:
        kernel._last_result = res
    return out



# revision 12
# speedup vs baseline: 1.2676x; 1.2676x over previous
"""Trainium2 Bass kernel for nn_Pndb_43344809951805 (scatter_memory).

Data-parallel over batch B=16 across 8 NeuronCores (2 batches/core).
Phase 1 writes the [Q,D] memory A (mean over B -> AllReduce), phase 2 reads it.
All big matmuls run in bf16 (full PE rate); residual path stays f32.
"""
import sys

sys.path.insert(0, "/opt/trn_rl_repo")

import numpy as np
import ml_dtypes

import concourse.bass as bass
import concourse.bacc as bacc
import concourse.mybir as mybir
import concourse.tile as tile
from concourse import masks
from concourse.bass_utils import run_bass_kernel_spmd

F32 = mybir.dt.float32
BF16 = mybir.dt.bfloat16
F8 = mybir.dt.float8e4
AF = mybir.ActivationFunctionType
ALU = mybir.AluOpType
DR = mybir.MatmulPerfMode.DoubleRow
BF = ml_dtypes.bfloat16
F8NP = mybir.dt.np(mybir.dt.float8e4)
RAW_SC = 16.0    # fp8 scale on raw/post_dec activations
W_SC = 32.0      # fp8 scale on Wk/Wo weights
INV_KSC = 1.0 / (RAW_SC * W_SC)

B, S, D, Q = 16, 2048, 1024, 64
NCORES = 8
BL = B // NCORES          # local batches per core
SBLK = 512                # s-block (matmul moving free dim)
NSB = S // SBLK           # 4 s-blocks per batch
NCH = S // 128            # 16 s-chunks per batch
NJ = D // 128             # 8 contraction chunks
NI = D // 128             # 8 output-dim chunks
CPB = SBLK // 128         # 4 chunks per s-block

_prog_cache = {}


def _build(bi_v: float, cgate_v: float, stage: str = "full"):
    nc = bacc.Bacc("TRN2", target_bir_lowering=False, debug=False,
                   enable_asserts=False, num_devices=NCORES)

    rawT_d = nc.dram_tensor("rawT", [BL, D, S], F8, kind="ExternalInput")
    rawN_d = nc.dram_tensor("rawN", [BL, S, D], BF16, kind="ExternalInput")
    pdT_d = nc.dram_tensor("pdT", [BL, D, S], F8, kind="ExternalInput")
    pdN_d = nc.dram_tensor("pdN", [BL, S, D], F32, kind="ExternalInput")
    wkT_d = nc.dram_tensor("wkT", [D, D], F8, kind="ExternalInput")
    woT_d = nc.dram_tensor("woT", [D, D], F8, kind="ExternalInput")
    qT1_d = nc.dram_tensor("qT1", [D, Q], BF16, kind="ExternalInput")
    qT2_d = nc.dram_tensor("qT2", [D, Q], BF16, kind="ExternalInput")
    bkT_d = nc.dram_tensor("bkT", [D, 1], F32, kind="ExternalInput")
    boT_d = nc.dram_tensor("boT", [D, 1], F32, kind="ExternalInput")
    wiB_d = nc.dram_tensor("wiB", [128, D], BF16, kind="ExternalInput")
    wu1B_d = nc.dram_tensor("wu1B", [128, D], F32, kind="ExternalInput")
    wu2B_d = nc.dram_tensor("wu2B", [Q, D], F32, kind="ExternalInput")
    out_d = nc.dram_tensor("out", [BL, S, D], F32, kind="ExternalOutput")

    with tile.TileContext(nc) as tc:
        with (
            tc.tile_pool(name="const", bufs=1) as cp,
            tc.tile_pool(name="dram", bufs=1, space="DRAM") as dram,
        ):
            # ---- constants; only wkT DMA'd up-front (first-MM critical) ----
            ident = cp.tile([128, 128], BF16, tag="ident")
            masks.make_identity(nc, ident[:])
            nbi = cp.tile([128, 1], F32, tag="nbi")
            nc.vector.memset(nbi[:], -bi_v)
            ncg = cp.tile([128, 1], F32, tag="ncg")
            nc.vector.memset(ncg[:], -cgate_v)

            wk8 = [cp.tile([128, 2, D], F8, tag=f"wk8{j}", name=f"wk8{j}")
                   for j in range(NJ // 2)]
            wo8 = [cp.tile([128, 2, D], F8, tag=f"wo8{j}", name=f"wo8{j}")
                   for j in range(NJ // 2)]
            qT1 = [cp.tile([128, Q], BF16, tag=f"qT1{j}", name=f"qT1{j}")
                   for j in range(NJ)]
            qT2 = [cp.tile([128, Q], BF16, tag=f"qT2{j}", name=f"qT2{j}")
                   for j in range(NJ)]
            bkT = [cp.tile([128, 1], F32, tag=f"bkT{j}", name=f"bkT{j}")
                   for j in range(NJ)]
            boT = [cp.tile([128, 1], F32, tag=f"boT{j}", name=f"boT{j}")
                   for j in range(NJ)]
            wiB = cp.tile([128, D], BF16, tag="wiB")
            wu1B = cp.tile([128, D], F32, tag="wu1B")
            wu2B = cp.tile([Q, D], F32, tag="wu2B")
            for j0 in range(NJ // 2):
                for h in range(2):
                    jj = 2 * j0 + h
                    nc.sync.dma_start(wk8[j0][:, h, :],
                                      wkT_d[jj * 128:(jj + 1) * 128, :])

            A_acc = cp.tile([Q, D], F32, tag="A_acc")
            A_f32 = cp.tile([Q, D], F32, tag="A_f32")
            A_bf = cp.tile([Q, D], BF16, tag="A_bf")
            awB = cp.tile([128, Q], BF16, tag="awB")
            scrA = cp.tile([Q, D], F32, tag="scrA")
            aw = cp.tile([Q, 1], F32, tag="aw")
            ar_in = dram.tile([Q + 1, D + 8], BF16)
            ar_out = dram.tile([Q + 1, D + 8], BF16)
            awz = cp.tile([Q, 8], BF16, tag="awz")
            nc.vector.memset(awz[:], 0.0)
            zrow = cp.tile([1, D + 8], BF16, tag="zrow")
            nc.vector.memset(zrow[:], 0.0)
            nc.gpsimd.dma_start(ar_in[0:Q, D:D + 8], awz[:])
            nc.gpsimd.dma_start(ar_in[Q:Q + 1, Q:D + 8], zrow[:, Q:D + 8])
            # phase-2 first-block data, prefetched during phase 1
            pdt0 = [cp.tile([128, 2, SBLK], F8, tag=f"pdt0_{j}",
                            name=f"pdt0_{j}") for j in range(NJ // 2)]
            pdn0 = [cp.tile([128, D], F32, tag=f"pdn0_{c}",
                            name=f"pdn0_{c}") for c in range(CPB)]


            # ================= PHASE 1 =================
            with (
                tc.tile_pool(name="p1", bufs=1) as p1,
                tc.tile_pool(name="p1ps", bufs=1, space="PSUM") as p1ps,
            ):
                def load_rawt(b, sb):
                    ts = []
                    for j0 in range(NJ // 2):
                        t = p1.tile([128, 2, SBLK], F8, tag=f"raw8_{j0}",
                                    name=f"raw8_{j0}", bufs=2)
                        for h in range(2):
                            jj = 2 * j0 + h
                            nc.sync.dma_start(
                                t[:, h, :],
                                rawT_d[b, jj * 128:(jj + 1) * 128,
                                       sb * SBLK:(sb + 1) * SBLK])
                        ts.append(t)
                    return ts

                for b in range(BL):
                    rawt = load_rawt(b, 0)
                    if b == 0:
                        nc.sync.dma_start(wiB[:], wiB_d[:])
                        for j in range(NJ):
                            sl = slice(j * 128, (j + 1) * 128)
                            nc.sync.dma_start(bkT[j][:], bkT_d[sl, :])
                            nc.sync.dma_start(qT1[j][:], qT1_d[sl, :])
                    U = p1.tile([Q, S], BF16, tag="U", bufs=2)
                    Zp = p1.tile([Q, NSB], F32, tag="Zp", bufs=2)
                    A_ps = p1ps.tile([Q, D], F32, tag="A_ps", bufs=1)

                    for sb in range(NSB):
                        # v-gate pre-pass + v for this s-block's chunks
                        Gg = p1.tile([128, CPB], F32, tag="Gg", bufs=2)
                        rns = []
                        for cc in range(CPB):
                            c = sb * CPB + cc
                            rn = p1.tile([128, D], BF16, tag=f"rawn{cc}",
                                         name=f"rawn{cc}", bufs=2)
                            nc.sync.dma_start(
                                rn[:], rawN_d[b, c * 128:(c + 1) * 128, :])
                            rns.append(rn)
                            scr = p1.tile([128, D], BF16, tag="scrb", bufs=2)
                            nc.vector.scalar_tensor_tensor(
                                scr[:], rn[:], 1.0, wiB[:],
                                ALU.mult, ALU.mult,
                                accum_out=Gg[:, cc:cc + 1])
                        nc.scalar.activation(Gg[:], Gg[:], AF.Exp,
                                             scale=-1.0, bias=nbi[:])
                        nc.vector.tensor_scalar_add(Gg[:], Gg[:], 1.0)
                        nc.vector.reciprocal(Gg[:], Gg[:])
                        vs = []
                        for cc in range(CPB):
                            v = p1.tile([128, D], BF16, tag=f"v{cc}",
                                        name=f"v{cc}", bufs=2)
                            nc.vector.tensor_scalar_mul(
                                v[:], rns[cc][:], Gg[:, cc:cc + 1])
                            vs.append(v)

                        nxt = load_rawt(b, sb + 1) if sb + 1 < NSB else None
                        if b == 0 and sb == 0:
                            # phase-2 weights: off the critical DMA path
                            for j in range(NJ):
                                sl = slice(j * 128, (j + 1) * 128)
                                nc.sync.dma_start(qT2[j][:], qT2_d[sl, :])
                                nc.sync.dma_start(boT[j][:], boT_d[sl, :])
                            for j0 in range(NJ // 2):
                                for h in range(2):
                                    jj = 2 * j0 + h
                                    nc.sync.dma_start(
                                        wo8[j0][:, h, :],
                                        woT_d[jj * 128:(jj + 1) * 128, :])
                            nc.sync.dma_start(wu1B[:], wu1B_d[:])
                            nc.sync.dma_start(wu2B[:], wu2B_d[:])
                        if b == 1 and sb == 0:
                            # prefetch phase-2 (b0, sb0) data
                            for j0 in range(NJ // 2):
                                for h in range(2):
                                    jj = 2 * j0 + h
                                    nc.sync.dma_start(
                                        pdt0[j0][:, h, :],
                                        pdT_d[0, jj * 128:(jj + 1) * 128,
                                              0:SBLK])
                            for c in range(CPB):
                                nc.sync.dma_start(
                                    pdn0[c][:],
                                    pdN_d[0, c * 128:(c + 1) * 128, :])

                        # kT matmuls + scores (software-pipelined by one i)
                        sc_ps = p1ps.tile([Q, SBLK], F32, tag="sc_ps", bufs=2)
                        kts = []
                        for i in range(NI):
                            isl = slice(i * 128, (i + 1) * 128)
                            k_ps = p1ps.tile([128, SBLK], F32, tag="k_ps",
                                             bufs=2)
                            for j0 in range(NJ // 2):
                                nc.tensor.matmul(
                                    k_ps[:], wk8[j0][:, :, isl], rawt[j0][:],
                                    start=(j0 == 0), stop=(j0 == NJ // 2 - 1),
                                    perf_mode=DR)
                            kt = p1.tile([128, SBLK], BF16, tag="kt", bufs=3)
                            nc.scalar.activation(kt[:], k_ps[:], AF.Identity,
                                                 scale=INV_KSC,
                                                 bias=bkT[i][:])
                            kts.append(kt)
                            if i >= 1:
                                nc.tensor.matmul(
                                    sc_ps[:], qT1[i - 1][:], kts[i - 1][:],
                                    start=(i - 1 == 0), stop=False,
                                    skip_group_check=True)
                        nc.tensor.matmul(
                            sc_ps[:], qT1[NI - 1][:], kts[NI - 1][:],
                            start=False, stop=True, skip_group_check=True)

                        ssl = slice(sb * SBLK, (sb + 1) * SBLK)
                        nc.scalar.activation(U[:, ssl], sc_ps[:], AF.Exp,
                                             accum_out=Zp[:, sb:sb + 1])
                        # transposes first (decoupled from A matmuls)
                        uts = []
                        for cc in range(CPB):
                            c = sb * CPB + cc
                            ut_ps = p1ps.tile([128, Q], BF16, tag="ut_ps",
                                              bufs=2)
                            nc.tensor.transpose(
                                ut_ps[:], U[:, c * 128:(c + 1) * 128],
                                ident[:Q, :Q])
                            ut = p1.tile([128, Q], BF16, tag="ut", bufs=6)
                            nc.scalar.copy(ut[:], ut_ps[:])
                            uts.append(ut)
                        for cc in range(CPB):
                            c = sb * CPB + cc
                            for h in range(2):
                                hsl = slice(h * 512, (h + 1) * 512)
                                nc.tensor.matmul(
                                    A_ps[:, hsl], uts[cc][:], vs[cc][:, hsl],
                                    start=(c == 0), stop=(c == NCH - 1),
                                    skip_group_check=True)
                        rawt = nxt

                    # A_acc += A_ps / (16 * Z)
                    Z1 = p1.tile([Q, 1], F32, tag="Z1", bufs=2)
                    nc.vector.tensor_reduce(Z1[:], Zp[:], mybir.AxisListType.X,
                                            ALU.add)
                    sA = p1.tile([Q, 1], F32, tag="sA", bufs=2)
                    nc.vector.reciprocal(sA[:], Z1[:])
                    nc.vector.tensor_scalar_mul(sA[:], sA[:], 1.0 / B)
                    if b == 0:
                        nc.vector.tensor_scalar_mul(A_acc[:], A_ps[:], sA[:])
                    else:
                        nc.vector.scalar_tensor_tensor(
                            A_acc[:], A_ps[:], sA[:], A_acc[:],
                            ALU.mult, ALU.add)

                # aw_local = A_acc . Wu2 rides along in the AllReduce
                nc.vector.scalar_tensor_tensor(
                    scrA[:], A_acc[:], 1.0, wu2B[:],
                    ALU.mult, ALU.mult, accum_out=aw[:])
                nc.gpsimd.dma_start(ar_in[0:Q, 0:D], A_acc[:])
                nc.gpsimd.dma_start(
                    ar_in[Q:Q + 1, 0:Q].rearrange("a b -> b a"), aw[:])

            # ---- AllReduce of partial A across the 8 cores ----
            if stage == "p1":
                nc.sync.dma_start(out_d[0, 0:Q, :], A_acc[:])
            else:
                if stage == "p2":
                    arr = ar_in
                else:
                    nc.gpsimd.collective_compute(
                        "AllReduce", ALU.add,
                        replica_groups=[list(range(NCORES))],
                        ins=[ar_in.opt()], outs=[ar_out.opt()],
                    )
                    arr = ar_out
                if stage == "ar":
                    nc.gpsimd.dma_start(A_f32[:], arr[0:Q, 0:D])
                nc.gpsimd.dma_start(A_bf[:], arr[0:Q, 0:D])
                nc.gpsimd.dma_start(
                    awB[:], arr[Q:Q + 1, 0:Q].broadcast_to([128, Q]))

            # ================= PHASE 2 =================
            if stage == "p1":
                pass  # skip phase 2
            else:
              with (
                  tc.tile_pool(name="p2", bufs=1) as p2,
                  tc.tile_pool(name="p2ps", bufs=1, space="PSUM") as p2ps,
              ):
                  def load_pdt(b, sb):
                      ts = []
                      for j0 in range(NJ // 2):
                          t = p2.tile([128, 2, SBLK], F8, tag=f"pdt{j0}",
                                      name=f"pdt{j0}", bufs=2)
                          for h in range(2):
                              jj = 2 * j0 + h
                              nc.sync.dma_start(
                                  t[:, h, :],
                                  pdT_d[b, jj * 128:(jj + 1) * 128,
                                        sb * SBLK:(sb + 1) * SBLK])
                          ts.append(t)
                      return ts

                  def emit_ko(pdt):
                      kot = []
                      for i in range(NI):
                          isl = slice(i * 128, (i + 1) * 128)
                          ko_ps = p2ps.tile([128, SBLK], F32, tag="ko_ps",
                                            bufs=2)
                          for j0 in range(NJ // 2):
                              nc.tensor.matmul(
                                  ko_ps[:], wo8[j0][:, :, isl], pdt[j0][:],
                                  start=(j0 == 0), stop=(j0 == NJ // 2 - 1),
                                  perf_mode=DR)
                          kt = p2.tile([128, SBLK], BF16, tag="kot",
                                       name="kot", bufs=16)
                          nc.scalar.activation(kt[:], ko_ps[:], AF.Identity,
                                               scale=INV_KSC,
                                               bias=boT[i][:])
                          kot.append(kt)
                      return kot

                  def emit_partB(st):
                      (Z2, u2s, b, sb, idx, G1, pdn) = st
                      G2 = p2.tile([128, CPB], F32, tag="G2", bufs=2)
                      SC = p2.tile([128, CPB], F32, tag="SC", bufs=2)
                      if G1 is None:
                          G1 = p2.tile([128, CPB], F32, tag="G1", bufs=2)
                          pdn = []
                          for cc in range(CPB):
                              c = sb * CPB + cc
                              if idx == 0:
                                  pn = pdn0[cc]
                              else:
                                  pn = p2.tile([128, D], F32,
                                               tag=f"pdn{cc}",
                                               name=f"pdn{cc}", bufs=2)
                                  nc.sync.dma_start(
                                      pn[:],
                                      pdN_d[b, c * 128:(c + 1) * 128, :])
                              pdn.append(pn)
                              scr = p2.tile([128, D], F32, tag="scrf",
                                            bufs=2)
                              nc.vector.scalar_tensor_tensor(
                                  scr[:], pn[:], 1.0, wu1B[:],
                                  ALU.mult, ALU.mult,
                                  accum_out=G1[:, cc:cc + 1])
                      for cc in range(CPB):
                          scr2 = p2.tile([128, Q], BF16, tag="scr2",
                                         bufs=2)
                          nc.vector.scalar_tensor_tensor(
                              scr2[:], u2s[cc][:], 1.0, awB[:],
                              ALU.mult, ALU.mult,
                              accum_out=G2[:, cc:cc + 1])
                      # gates: sc = sigmoid(g1 + g2/Z + cg) / Z
                      rz = p2.tile([128, CPB], F32, tag="rz", bufs=2)
                      nc.vector.reciprocal(rz[:], Z2[:])
                      t4 = p2.tile([128, CPB], F32, tag="t4", bufs=2)
                      nc.vector.tensor_mul(t4[:], G2[:], rz[:])
                      nc.vector.tensor_add(t4[:], t4[:], G1[:])
                      e4 = p2.tile([128, CPB], F32, tag="e4", bufs=2)
                      nc.scalar.activation(e4[:], t4[:], AF.Exp,
                                           scale=-1.0, bias=ncg[:])
                      nc.vector.tensor_scalar_add(e4[:], e4[:], 1.0)
                      nc.vector.reciprocal(e4[:], e4[:])
                      nc.vector.tensor_mul(SC[:], e4[:], rz[:])
                      # transposes, then A2 matmuls + fused residual
                      ut2s = []
                      for cc in range(CPB):
                          ut2_ps = p2ps.tile([Q, 128], BF16, tag="ut2_ps",
                                             bufs=2)
                          nc.tensor.transpose(ut2_ps[:], u2s[cc][:],
                                              ident[:, :])
                          ut2 = p2.tile([Q, 128], BF16, tag="ut2", bufs=6)
                          nc.scalar.copy(ut2[:], ut2_ps[:])
                          ut2s.append(ut2)
                      for cc in range(CPB):
                          c = sb * CPB + cc
                          outt = p2.tile([128, D], F32, tag="outt", bufs=3)
                          for h in range(2):
                              hsl = slice(h * 512, (h + 1) * 512)
                              a2_ps = p2ps.tile([128, 512], F32,
                                                tag="a2_ps", bufs=2)
                              nc.tensor.matmul(a2_ps[:], ut2s[cc][:],
                                               A_bf[:, hsl],
                                               start=True, stop=True)
                              nc.vector.scalar_tensor_tensor(
                                  outt[:, hsl], a2_ps[:],
                                  SC[:, cc:cc + 1],
                                  pdn[cc][:, hsl], ALU.mult, ALU.add)
                          nc.sync.dma_start(
                              out_d[b, c * 128:(c + 1) * 128, :], outt[:])

                  all_sb = [(b, sb) for b in range(BL) for sb in range(NSB)]
                  pdt_cur = pdt0
                  pending = []
                  for idx, (b, sb) in enumerate(all_sb):
                      kot = emit_ko(pdt_cur)
                      pdt_nxt = (load_pdt(*all_sb[idx + 1])
                                 if idx + 1 < len(all_sb) else None)
                      # part A: s2 + exp per chunk (A-independent)
                      Z2 = p2.tile([128, CPB], F32, tag="Z2", bufs=4)
                      u2s = []
                      for cc in range(CPB):
                          c = sb * CPB + cc
                          s2_ps = p2ps.tile([128, Q], F32, tag="s2_ps",
                                            bufs=2)
                          for i in range(NI):
                              nc.tensor.matmul(
                                  s2_ps[:],
                                  kot[i][:, cc * 128:(cc + 1) * 128],
                                  qT2[i][:],
                                  start=(i == 0), stop=(i == NI - 1),
                                  skip_group_check=True)
                          u2 = p2.tile([128, Q], BF16, tag=f"u2{cc}",
                                       name=f"u2{cc}", bufs=4)
                          nc.scalar.activation(u2[:], s2_ps[:], AF.Exp,
                                               accum_out=Z2[:, cc:cc + 1])
                          u2s.append(u2)

                      G1e = None
                      pdne = None
                      if idx >= len(all_sb) - 2:
                          G1e = p2.tile([128, CPB], F32, tag="G1L", bufs=2)
                          pdne = []
                          for cc in range(CPB):
                              c = sb * CPB + cc
                              pn = p2.tile([128, D], F32, tag=f"pdnL{cc}",
                                           name=f"pdnL{cc}", bufs=2)
                              nc.sync.dma_start(
                                  pn[:],
                                  pdN_d[b, c * 128:(c + 1) * 128, :])
                              pdne.append(pn)
                              scr = p2.tile([128, D], F32, tag="scrf",
                                            bufs=2)
                              nc.vector.scalar_tensor_tensor(
                                  scr[:], pn[:], 1.0, wu1B[:],
                                  ALU.mult, ALU.mult,
                                  accum_out=G1e[:, cc:cc + 1])

                      depth = 3 if idx <= 4 else 2
                      while len(pending) >= depth:
                          emit_partB(pending.pop(0))
                      pending.append((Z2, u2s, b, sb, idx, G1e, pdne))
                      pdt_cur = pdt_nxt
                  for st in pending:
                      emit_partB(st)

            if stage == "ar":
                nc.sync.dma_start(out_d[0, 0:Q, :], A_f32[:])
                nc.gpsimd.dma_start(out_d[0, 128:256, 0:Q], awB[:])
    nc.compile()
    return nc


def _get_prog(bi_v, cgate_v):
    key = (round(bi_v, 9), round(cgate_v, 9))
    if key not in _prog_cache:
        _prog_cache[key] = _build(bi_v, cgate_v)
    return _prog_cache[key]


def kernel(raw, post_dec, mask, questions, Wk, bk, Wi, bi, Wo, bo,
           Wu1, bu1, Wu2, bu2, b1, _trace=False):
    raw = np.asarray(raw, dtype=np.float32)
    post_dec = np.asarray(post_dec, dtype=np.float32)
    questions = np.asarray(questions, dtype=np.float32)
    Wk = np.asarray(Wk, dtype=np.float32)
    Wo = np.asarray(Wo, dtype=np.float32)

    bi_v = float(np.asarray(bi).reshape(-1)[0])
    cgate_v = float(np.asarray(bu1).reshape(-1)[0]
                    + np.asarray(bu2).reshape(-1)[0]
                    + np.asarray(b1).reshape(-1)[0])
    nc = _get_prog(bi_v, cgate_v)

    inv_sqrt_d = np.float32(1.0 / np.sqrt(D))
    inv_sqrt_q = np.float32(1.0 / np.sqrt(Q))

    def to_f8(x):
        return np.clip(x, -240.0, 240.0).astype(F8NP)

    wkT = to_f8(np.ascontiguousarray(Wk.T) * W_SC)
    woT = to_f8(np.ascontiguousarray(Wo.T) * W_SC)
    qT1 = np.ascontiguousarray(questions.T * inv_sqrt_d).astype(BF)
    qT2 = np.ascontiguousarray(questions.T * inv_sqrt_q).astype(BF)
    bkT = np.ascontiguousarray(np.asarray(bk, np.float32).reshape(D, 1))
    boT = np.ascontiguousarray(np.asarray(bo, np.float32).reshape(D, 1))
    wiB = np.ascontiguousarray(
        np.broadcast_to(np.asarray(Wi, np.float32).reshape(1, D), (128, D))
    ).astype(BF)
    wu1B = np.ascontiguousarray(
        np.broadcast_to(np.asarray(Wu1, np.float32).reshape(1, D), (128, D)))
    wu2B = np.ascontiguousarray(
        np.broadcast_to(np.asarray(Wu2, np.float32).reshape(1, D), (Q, D)))

    in_maps = []
    for r in range(NCORES):
        bs = slice(r * BL, (r + 1) * BL)
        rawT = to_f8(np.ascontiguousarray(
            raw[bs].transpose(0, 2, 1)) * RAW_SC)
        rawN = np.ascontiguousarray(raw[bs]).astype(BF)
        pdT = to_f8(np.ascontiguousarray(
            post_dec[bs].transpose(0, 2, 1)) * RAW_SC)
        pdN = np.ascontiguousarray(post_dec[bs])
        in_maps.append({
            "rawT": rawT, "rawN": rawN, "pdT": pdT, "pdN": pdN,
            "wkT": wkT, "woT": woT, "qT1": qT1, "qT2": qT2,
            "bkT": bkT, "boT": boT, "wiB": wiB, "wu1B": wu1B, "wu2B": wu2B,
        })

    res = run_bass_kernel_spmd(nc, in_maps, core_ids=list(range(NCORES)),
                               trace=_trace)
    out = np.concatenate([res.results[r]["out"] for r in range(NCORES)],
                         axis=0)
    if _trace:
        kernel._last_result = res
    return out



# revision 13
# speedup vs baseline: 1.3652x; 1.0771x over previous
"""Trainium2 Bass kernel for nn_Pndb_43344809951805 (scatter_memory).

Data-parallel over batch B=16 across 8 NeuronCores (2 batches/core).
Phase 1 writes the [Q,D] memory A (mean over B -> AllReduce), phase 2 reads it.
Big GEMMs (k = raw@Wk.T, ko = pd@Wo.T) run fp8e4 DoubleRow (K=256/instr,
2x PE rate); attention matmuls bf16. Phase 2 is transposeless: s2 computed
in [q,s] layout, Z2/G2 via a ones|aw matmul, A2 directly from U2 chunks.
The v-gate is folded into the transposed-U scale so A's rhs is rawN itself.
"""
import sys

sys.path.insert(0, "/opt/trn_rl_repo")

import numpy as np
import ml_dtypes

import concourse.bass as bass
import concourse.bacc as bacc
import concourse.mybir as mybir
import concourse.tile as tile
from concourse import masks
from concourse.bass_utils import run_bass_kernel_spmd

F32 = mybir.dt.float32
BF16 = mybir.dt.bfloat16
F8 = mybir.dt.float8e4
AF = mybir.ActivationFunctionType
ALU = mybir.AluOpType
DR = mybir.MatmulPerfMode.DoubleRow
BF = ml_dtypes.bfloat16
F8NP = mybir.dt.np(mybir.dt.float8e4)
RAW_SC = 16.0    # fp8 scale on raw/post_dec activations
W_SC = 32.0      # fp8 scale on Wk/Wo weights
INV_KSC = 1.0 / (RAW_SC * W_SC)

B, S, D, Q = 16, 2048, 1024, 64
NCORES = 8
BL = B // NCORES          # local batches per core
SBLK = 512                # s-block (matmul moving free dim)
NSB = S // SBLK           # 4 s-blocks per batch
NCH = S // 128            # 16 s-chunks per batch
NJ = D // 128             # 8 contraction chunks
NI = D // 128             # 8 output-dim chunks
CPB = SBLK // 128         # 4 chunks per s-block

_prog_cache = {}


def _build(bi_v: float, cgate_v: float, stage: str = "full"):
    nc = bacc.Bacc("TRN2", target_bir_lowering=False, debug=False,
                   enable_asserts=False, num_devices=NCORES)

    rawT_d = nc.dram_tensor("rawT", [BL, D, S], F8, kind="ExternalInput")
    rawN_d = nc.dram_tensor("rawN", [BL, S, D], BF16, kind="ExternalInput")
    pdT_d = nc.dram_tensor("pdT", [BL, D, S], F8, kind="ExternalInput")
    pdN_d = nc.dram_tensor("pdN", [BL, S, D], BF16, kind="ExternalInput")
    wkT_d = nc.dram_tensor("wkT", [D, D], F8, kind="ExternalInput")
    woT_d = nc.dram_tensor("woT", [D, D], F8, kind="ExternalInput")
    qT1_d = nc.dram_tensor("qT1", [D, Q], BF16, kind="ExternalInput")
    qT2_d = nc.dram_tensor("qT2", [D, Q], BF16, kind="ExternalInput")
    bkT_d = nc.dram_tensor("bkT", [D, 1], F32, kind="ExternalInput")
    boT_d = nc.dram_tensor("boT", [D, 1], F32, kind="ExternalInput")
    wiB_d = nc.dram_tensor("wiB", [128, D], BF16, kind="ExternalInput")
    wu1B_d = nc.dram_tensor("wu1B", [128, D], BF16, kind="ExternalInput")
    wu2B_d = nc.dram_tensor("wu2B", [Q, D], F32, kind="ExternalInput")
    out_d = nc.dram_tensor("out", [BL, S, D], F32, kind="ExternalOutput")

    # [D, X] -> [128, NJ, X] paired-chunk view for single-descriptor DMA
    def chunked(ap):
        return ap.rearrange("(j p) x -> p j x", p=128)

    with tile.TileContext(nc) as tc:
        with (
            tc.tile_pool(name="const", bufs=1) as cp,
            tc.tile_pool(name="dram", bufs=1, space="DRAM") as dram,
        ):
            ident = cp.tile([128, 128], BF16, tag="ident")
            masks.make_identity(nc, ident[:])
            nbi = cp.tile([128, 1], F32, tag="nbi")
            nc.vector.memset(nbi[:], -bi_v)
            ncg = cp.tile([128, 1], F32, tag="ncg")
            nc.vector.memset(ncg[:], -cgate_v)

            wk8 = cp.tile([128, NJ, D], F8, tag="wk8")
            wo8 = cp.tile([128, NJ, D], F8, tag="wo8")
            qT1 = cp.tile([128, NJ, Q], BF16, tag="qT1")
            qT2 = cp.tile([128, NJ, Q], BF16, tag="qT2")
            bkT = cp.tile([128, NJ], F32, tag="bkT")
            boT = cp.tile([128, NJ], F32, tag="boT")
            wiB = cp.tile([128, D], BF16, tag="wiB")
            wu1B = cp.tile([128, D], BF16, tag="wu1B")
            wu2B = cp.tile([Q, D], F32, tag="wu2B")
            # phase-2 pdT, fully prefetched during phase 1
            pdt_all = [cp.tile([128, NJ, S], F8, tag=f"pdtA{b}",
                               name=f"pdtA{b}") for b in range(BL)]

            # critical-path weights first
            nc.sync.dma_start(wk8[:], chunked(wkT_d[:, :]))
            nc.sync.dma_start(qT1[:], chunked(qT1_d[:, :]))
            nc.sync.dma_start(bkT[:],
                              bkT_d[:, :].rearrange("(i p) o -> p (i o)",
                                                    p=128))
            nc.sync.dma_start(wiB[:], wiB_d[:])

            A_acc = cp.tile([Q, D], F32, tag="A_acc")
            A_f32 = cp.tile([Q, D], F32, tag="A_f32")
            A_bf = cp.tile([Q, D], BF16, tag="A_bf")
            awo = cp.tile([Q, 2], BF16, tag="awo")
            nc.vector.memset(awo[:, 0:1], 1.0)
            scrA = cp.tile([Q, D], F32, tag="scrA")
            aw = cp.tile([Q, 1], F32, tag="aw")
            ar_in = dram.tile([Q + 1, D + 8], BF16)
            ar_out = dram.tile([Q + 1, D + 8], BF16)
            awz = cp.tile([Q, 8], BF16, tag="awz")
            nc.vector.memset(awz[:], 0.0)
            zrow = cp.tile([1, D + 8], BF16, tag="zrow")
            nc.vector.memset(zrow[:], 0.0)
            nc.gpsimd.dma_start(ar_in[0:Q, D:D + 8], awz[:])
            nc.gpsimd.dma_start(ar_in[Q:Q + 1, Q:D + 8], zrow[:, Q:D + 8])

            # ================= PHASE 1 =================
            with (
                tc.tile_pool(name="p1", bufs=1) as p1,
                tc.tile_pool(name="p1ps", bufs=1, space="PSUM") as p1ps,
            ):
                def load_raw8(b, sb):
                    t = p1.tile([128, NJ, SBLK], F8, tag="raw8",
                                name="raw8", bufs=2)
                    nc.sync.dma_start(
                        t[:], chunked(rawT_d[b])[
                            :, :, sb * SBLK:(sb + 1) * SBLK])
                    return t

                def load_rns(b, sb):
                    t = p1.tile([128, CPB, D], BF16, tag="rns",
                                name="rns", bufs=2)
                    nc.sync.dma_start(
                        t[:],
                        rawN_d[b, sb * SBLK:(sb + 1) * SBLK, :].rearrange(
                            "(c p) d -> p c d", p=128))
                    return t

                raw8_cur = load_raw8(0, 0)
                rns_cur = load_rns(0, 0)

                all_p1 = [(b, sb) for b in range(BL) for sb in range(NSB)]
                for b in range(BL):
                    Zp = p1.tile([Q, NSB], F32, tag="Zp", bufs=2)
                    A_ps = p1ps.tile([Q, D], F32, tag="A_ps", bufs=1)

                    for sb in range(NSB):
                        idx = b * NSB + sb
                        # ---- v-gate: Gg = sigmoid(raw . Wi + bi) ----
                        Gg = p1.tile([128, CPB], F32, tag="Gg", bufs=2)
                        for cc in range(CPB):
                            scr = p1.tile([128, D], BF16, tag="scrb", bufs=2)
                            nc.vector.scalar_tensor_tensor(
                                scr[:], rns_cur[:, cc, :], 1.0, wiB[:],
                                ALU.mult, ALU.mult,
                                accum_out=Gg[:, cc:cc + 1])
                        nc.scalar.activation(Gg[:], Gg[:], AF.Exp,
                                             scale=-1.0, bias=nbi[:])
                        nc.vector.tensor_scalar_add(Gg[:], Gg[:], 1.0)
                        nc.vector.reciprocal(Gg[:], Gg[:])

                        # prefetches
                        if idx + 1 < len(all_p1):
                            nb, nsb = all_p1[idx + 1]
                            raw8_nxt = load_raw8(nb, nsb)
                            rns_nxt = load_rns(nb, nsb)
                        else:
                            raw8_nxt = rns_nxt = None
                        if b == 0 and sb == 0:
                            # phase-2 weights: off the critical DMA path
                            nc.sync.dma_start(wo8[:], chunked(woT_d[:, :]))
                            nc.sync.dma_start(qT2[:], chunked(qT2_d[:, :]))
                            nc.sync.dma_start(
                                boT[:],
                                boT_d[:, :].rearrange("(i p) o -> p (i o)",
                                                      p=128))
                            nc.sync.dma_start(wu1B[:], wu1B_d[:])
                            nc.sync.dma_start(wu2B[:], wu2B_d[:])
                        # spread the phase-2 pdT prefetch over 4 slots
                        if idx in (1, 2, 3, 4):
                            pb, ph = divmod(idx - 1, 2)
                            nc.sync.dma_start(
                                pdt_all[pb][:, 4 * ph:4 * (ph + 1), :],
                                chunked(pdT_d[pb])[:, 4 * ph:4 * (ph + 1), :])

                        # ---- k GEMM (fp8 DR) + scores, sw-pipelined ----
                        sc_ps = p1ps.tile([Q, SBLK], F32, tag="sc_ps", bufs=2)
                        kts = []
                        for i in range(NI):
                            isl = slice(i * 128, (i + 1) * 128)
                            k_ps = p1ps.tile([128, SBLK], F32, tag="k_ps",
                                             bufs=2)
                            for j0 in range(NJ // 2):
                                nc.tensor.matmul(
                                    k_ps[:],
                                    wk8[:, 2 * j0:2 * j0 + 2, isl],
                                    raw8_cur[:, 2 * j0:2 * j0 + 2, :],
                                    start=(j0 == 0),
                                    stop=(j0 == NJ // 2 - 1),
                                    perf_mode=DR)
                            kt = p1.tile([128, SBLK], BF16, tag="kt", bufs=3)
                            nc.scalar.activation(kt[:], k_ps[:], AF.Identity,
                                                 scale=INV_KSC,
                                                 bias=bkT[:, i:i + 1])
                            kts.append(kt)
                            if i >= 1:
                                nc.tensor.matmul(
                                    sc_ps[:], qT1[:, i - 1, :], kts[i - 1][:],
                                    start=(i - 1 == 0), stop=False,
                                    skip_group_check=True)
                        nc.tensor.matmul(
                            sc_ps[:], qT1[:, NI - 1, :], kts[NI - 1][:],
                            start=False, stop=True, skip_group_check=True)

                        # ---- exp, transpose, scale by gate ----
                        U = p1.tile([Q, SBLK], BF16, tag="U", bufs=2)
                        nc.scalar.activation(U[:], sc_ps[:], AF.Exp,
                                             accum_out=Zp[:, sb:sb + 1])
                        ut_ps = p1ps.tile([128, CPB, Q], BF16, tag="ut_ps",
                                          bufs=2)
                        for cc in range(CPB):
                            nc.tensor.transpose(
                                ut_ps[:, cc, :],
                                U[:, cc * 128:(cc + 1) * 128],
                                ident[:Q, :Q])
                        uts = p1.tile([128, CPB, Q], BF16, tag="uts", bufs=2)
                        for cc in range(CPB):
                            nc.scalar.activation(uts[:, cc, :],
                                                 ut_ps[:, cc, :], AF.Copy,
                                                 scale=Gg[:, cc:cc + 1])
                        # ---- A += (U*g).T @ rawN over chunks ----
                        for cc in range(CPB):
                            c = sb * CPB + cc
                            for h in range(2):
                                hsl = slice(h * 512, (h + 1) * 512)
                                nc.tensor.matmul(
                                    A_ps[:, hsl], uts[:, cc, :],
                                    rns_cur[:, cc, hsl],
                                    start=(c == 0), stop=(c == NCH - 1),
                                    skip_group_check=True)
                        raw8_cur, rns_cur = raw8_nxt, rns_nxt

                    # A_acc += A_ps / (16 * Z)
                    Z1 = p1.tile([Q, 1], F32, tag="Z1", bufs=2)
                    nc.vector.tensor_reduce(Z1[:], Zp[:], mybir.AxisListType.X,
                                            ALU.add)
                    sA = p1.tile([Q, 1], F32, tag="sA", bufs=2)
                    nc.vector.reciprocal(sA[:], Z1[:])
                    nc.vector.tensor_scalar_mul(sA[:], sA[:], 1.0 / B)
                    if b == 0:
                        nc.vector.tensor_scalar_mul(A_acc[:], A_ps[:], sA[:])
                    else:
                        nc.vector.scalar_tensor_tensor(
                            A_acc[:], A_ps[:], sA[:], A_acc[:],
                            ALU.mult, ALU.add)

                # aw_local = A_acc . Wu2 rides along in the AllReduce
                nc.vector.scalar_tensor_tensor(
                    scrA[:], A_acc[:], 1.0, wu2B[:],
                    ALU.mult, ALU.mult, accum_out=aw[:])
                nc.gpsimd.dma_start(ar_in[0:Q, 0:D], A_acc[:])
                nc.gpsimd.dma_start(
                    ar_in[Q:Q + 1, 0:Q].rearrange("a b -> b a"), aw[:])

            # ---- AllReduce of partial A across the 8 cores ----
            if stage == "p1":
                nc.sync.dma_start(out_d[0, 0:Q, :], A_acc[:])
            else:
                if stage == "p2":
                    arr = ar_in
                else:
                    nc.gpsimd.collective_compute(
                        "AllReduce", ALU.add,
                        replica_groups=[list(range(NCORES))],
                        ins=[ar_in.opt()], outs=[ar_out.opt()],
                    )
                    arr = ar_out
                if stage == "ar":
                    nc.gpsimd.dma_start(A_f32[:], arr[0:Q, 0:D])
                nc.gpsimd.dma_start(A_bf[:], arr[0:Q, 0:D])
                nc.gpsimd.dma_start(
                    awo[0:Q, 1:2], arr[Q:Q + 1, 0:Q].rearrange("a b -> b a"))

            # ================= PHASE 2 =================
            if stage == "p1":
                pass  # skip phase 2
            else:
              with (
                  tc.tile_pool(name="p2", bufs=1) as p2,
                  tc.tile_pool(name="p2ps", bufs=1, space="PSUM") as p2ps,
              ):
                  def part_a(b, sb):
                      ssl = slice(sb * SBLK, (sb + 1) * SBLK)
                      pdn = p2.tile([128, CPB, D], BF16, tag="pdn",
                                    name="pdn", bufs=4)
                      nc.sync.dma_start(
                          pdn[:],
                          pdN_d[b, ssl, :].rearrange("(c p) d -> p c d",
                                                     p=128))
                      # ko GEMM (fp8 DR) + s2 in [q, s] layout, sw-pipelined
                      sc2_ps = p2ps.tile([Q, SBLK], F32, tag="s2_ps", bufs=2)
                      kot = []
                      for i in range(NI):
                          isl = slice(i * 128, (i + 1) * 128)
                          ko_ps = p2ps.tile([128, SBLK], F32, tag="ko_ps",
                                            bufs=2)
                          for j0 in range(NJ // 2):
                              nc.tensor.matmul(
                                  ko_ps[:],
                                  wo8[:, 2 * j0:2 * j0 + 2, isl],
                                  pdt_all[b][:, 2 * j0:2 * j0 + 2, ssl],
                                  start=(j0 == 0),
                                  stop=(j0 == NJ // 2 - 1),
                                  perf_mode=DR)
                          kt = p2.tile([128, SBLK], BF16, tag="kot",
                                       name="kot", bufs=3)
                          nc.scalar.activation(kt[:], ko_ps[:], AF.Identity,
                                               scale=INV_KSC,
                                               bias=boT[:, i:i + 1])
                          kot.append(kt)
                          if i >= 1:
                              nc.tensor.matmul(
                                  sc2_ps[:], qT2[:, i - 1, :], kot[i - 1][:],
                                  start=(i - 1 == 0), stop=False,
                                  skip_group_check=True)
                      nc.tensor.matmul(
                          sc2_ps[:], qT2[:, NI - 1, :], kot[NI - 1][:],
                          start=False, stop=True, skip_group_check=True)
                      U2 = p2.tile([Q, SBLK], BF16, tag="U2", bufs=5)
                      nc.scalar.activation(U2[:], sc2_ps[:], AF.Exp)
                      # G1 = pd . Wu1 per chunk
                      G1t = p2.tile([128, CPB], F32, tag="G1t", bufs=5)
                      for cc in range(CPB):
                          scr2 = p2.tile([128, D], BF16, tag="scr2", bufs=2)
                          nc.vector.scalar_tensor_tensor(
                              scr2[:], pdn[:, cc, :], 1.0, wu1B[:],
                              ALU.mult, ALU.mult,
                              accum_out=G1t[:, cc:cc + 1])
                      return (U2, G1t, pdn, b, sb)

                  def part_b(st):
                      (U2, G1t, pdn, b, sb) = st
                      # Z2 and G2 in one matmul against [ones | aw]
                      zg_ps = p2ps.tile([128, 2 * CPB], F32, tag="zg",
                                        bufs=2)
                      for cc in range(CPB):
                          nc.tensor.matmul(
                              zg_ps[:, 2 * cc:2 * cc + 2],
                              U2[:, cc * 128:(cc + 1) * 128], awo[:],
                              start=(cc == 0), stop=(cc == CPB - 1),
                              skip_group_check=True)
                      # SC = sigmoid(G1 + G2/Z2 + cg) / Z2
                      rz = p2.tile([128, CPB], F32, tag="rz", bufs=2)
                      nc.vector.reciprocal(rz[:], zg_ps[:, 0::2])
                      t4 = p2.tile([128, CPB], F32, tag="t4", bufs=2)
                      nc.vector.tensor_mul(t4[:], zg_ps[:, 1::2], rz[:])
                      nc.vector.tensor_add(t4[:], t4[:], G1t[:])
                      e4 = p2.tile([128, CPB], F32, tag="e4", bufs=2)
                      nc.scalar.activation(e4[:], t4[:], AF.Exp,
                                           scale=-1.0, bias=ncg[:])
                      nc.vector.tensor_scalar_add(e4[:], e4[:], 1.0)
                      nc.vector.reciprocal(e4[:], e4[:])
                      SC = p2.tile([128, CPB], F32, tag="SC", bufs=2)
                      nc.vector.tensor_mul(SC[:], e4[:], rz[:])
                      # A2 + fused residual
                      for cc in range(CPB):
                          c = sb * CPB + cc
                          outt = p2.tile([128, D], F32, tag="outt", bufs=3)
                          for h in range(2):
                              hsl = slice(h * 512, (h + 1) * 512)
                              a2_ps = p2ps.tile([128, 512], F32,
                                                tag="a2_ps", bufs=2)
                              nc.tensor.matmul(
                                  a2_ps[:], U2[:, cc * 128:(cc + 1) * 128],
                                  A_bf[:, hsl], start=True, stop=True)
                              nc.vector.scalar_tensor_tensor(
                                  outt[:, hsl], a2_ps[:],
                                  SC[:, cc:cc + 1],
                                  pdn[:, cc, hsl], ALU.mult, ALU.add)
                          nc.gpsimd.dma_start(
                              out_d[b, c * 128:(c + 1) * 128, :], outt[:])

                  all_sb = [(b, sb) for b in range(BL) for sb in range(NSB)]
                  pending = []
                  for idx, (b, sb) in enumerate(all_sb):
                      st = part_a(b, sb)
                      depth = 3 if idx <= 4 else 2
                      while len(pending) >= depth:
                          part_b(pending.pop(0))
                      pending.append(st)
                  for st in pending:
                      part_b(st)

            if stage == "ar":
                nc.sync.dma_start(out_d[0, 0:Q, :], A_f32[:])
    nc.compile()
    return nc


def _get_prog(bi_v, cgate_v):
    key = (round(bi_v, 9), round(cgate_v, 9))
    if key not in _prog_cache:
        _prog_cache[key] = _build(bi_v, cgate_v)
    return _prog_cache[key]


def kernel(raw, post_dec, mask, questions, Wk, bk, Wi, bi, Wo, bo,
           Wu1, bu1, Wu2, bu2, b1, _trace=False):
    raw = np.asarray(raw, dtype=np.float32)
    post_dec = np.asarray(post_dec, dtype=np.float32)
    questions = np.asarray(questions, dtype=np.float32)
    Wk = np.asarray(Wk, dtype=np.float32)
    Wo = np.asarray(Wo, dtype=np.float32)

    bi_v = float(np.asarray(bi).reshape(-1)[0])
    cgate_v = float(np.asarray(bu1).reshape(-1)[0]
                    + np.asarray(bu2).reshape(-1)[0]
                    + np.asarray(b1).reshape(-1)[0])
    nc = _get_prog(bi_v, cgate_v)

    inv_sqrt_d = np.float32(1.0 / np.sqrt(D))
    inv_sqrt_q = np.float32(1.0 / np.sqrt(Q))

    def to_f8(x):
        return np.clip(x, -240.0, 240.0).astype(F8NP)

    wkT = to_f8(np.ascontiguousarray(Wk.T) * W_SC)
    woT = to_f8(np.ascontiguousarray(Wo.T) * W_SC)
    qT1 = np.ascontiguousarray(questions.T * inv_sqrt_d).astype(BF)
    qT2 = np.ascontiguousarray(questions.T * inv_sqrt_q).astype(BF)
    bkT = np.ascontiguousarray(np.asarray(bk, np.float32).reshape(D, 1))
    boT = np.ascontiguousarray(np.asarray(bo, np.float32).reshape(D, 1))
    wiB = np.ascontiguousarray(
        np.broadcast_to(np.asarray(Wi, np.float32).reshape(1, D), (128, D))
    ).astype(BF)
    wu1B = np.ascontiguousarray(
        np.broadcast_to(np.asarray(Wu1, np.float32).reshape(1, D),
                        (128, D))).astype(BF)
    wu2B = np.ascontiguousarray(
        np.broadcast_to(np.asarray(Wu2, np.float32).reshape(1, D), (Q, D)))

    in_maps = []
    for r in range(NCORES):
        bs = slice(r * BL, (r + 1) * BL)
        rawT = to_f8(np.ascontiguousarray(
            raw[bs].transpose(0, 2, 1)) * RAW_SC)
        rawN = np.ascontiguousarray(raw[bs]).astype(BF)
        pdT = to_f8(np.ascontiguousarray(
            post_dec[bs].transpose(0, 2, 1)) * RAW_SC)
        pdN = np.ascontiguousarray(post_dec[bs]).astype(BF)
        in_maps.append({
            "rawT": rawT, "rawN": rawN, "pdT": pdT, "pdN": pdN,
            "wkT": wkT, "woT": woT, "qT1": qT1, "qT2": qT2,
            "bkT": bkT, "boT": boT, "wiB": wiB, "wu1B": wu1B, "wu2B": wu2B,
        })

    res = run_bass_kernel_spmd(nc, in_maps, core_ids=list(range(NCORES)),
                               trace=_trace)
    out = np.concatenate([res.results[r]["out"] for r in range(NCORES)],
                         axis=0)
    if _trace:
        kernel._last_result = res
    return out


# revision 19
# speedup vs baseline: 2.1727x; 1.5914x over previous
"""Trainium2 Bass kernel for nn_Pndb_43344809951805 (scatter_memory).

Data-parallel over batch B=16 across 8 NeuronCores (2 batches/core).

Key algebraic restructure: scores = q@(raw@Wk.T).T = (q@Wk)@raw.T, so the
[D,D] k/ko GEMMs are replaced by tiny host-side [Q,D] projections (qWk, qWo).
The v-gate (raw.Wi) and u-gate (pd.Wu1) dot products ride along as a 65th
output row of the scores/s2 matmuls with sign-flipped weights, so one Exp
activation yields both exp(scores) and exp(-gate_arg). Phase 1 recovers the
gate row via the 65-wide PE transposes; phase 2 recovers it via a third
column of the Z2/G2 ones|aw matmul. All big matmuls are fp8e4 DoubleRow
(K=256/instruction). A = mean over B needs an AllReduce of the per-core
partial [Q,D] A (plus the A.Wu2 row), overlapped with phase-2 partA work.
"""
import sys

sys.path.insert(0, "/opt/trn_rl_repo")

import numpy as np
import ml_dtypes

import concourse.bass as bass
import concourse.bacc as bacc
import concourse.mybir as mybir
import concourse.tile as tile
from concourse import masks
from concourse.bass_utils import run_bass_kernel_spmd

F32 = mybir.dt.float32
BF16 = mybir.dt.bfloat16
F8 = mybir.dt.float8e4
AF = mybir.ActivationFunctionType
ALU = mybir.AluOpType
DR = mybir.MatmulPerfMode.DoubleRow
BF = ml_dtypes.bfloat16
F8NP = mybir.dt.np(mybir.dt.float8e4)
RAW_SC = 16.0    # fp8 scale on raw/post_dec activations
QW_SC = 16.0     # fp8 scale on qWk/qWo/-Wi/-Wu1 stationary weights
SC_SC = RAW_SC * QW_SC          # psum score scale (256)
INV_SC = 1.0 / SC_SC

B, S, D, Q = 16, 2048, 1024, 64
Q1 = Q + 1                # extra gate row
NCORES = 8
BL = B // NCORES          # local batches per core
SBLK = 512                # s-block (matmul moving free dim)
NSB = S // SBLK           # 4 s-blocks per batch
NCH = S // 128            # 16 s-chunks per batch
NJ = D // 128             # 8 contraction chunks
CPB = SBLK // 128         # 4 chunks per s-block

_prog_cache = {}


def _build(stage: str = "full"):
    nc = bacc.Bacc("TRN2", target_bir_lowering=False, debug=False,
                   enable_asserts=False, num_devices=NCORES)

    rawT_d = nc.dram_tensor("rawT", [BL, D, S], F8, kind="ExternalInput")
    rawN_d = nc.dram_tensor("rawN", [BL, S, D], F8, kind="ExternalInput")
    pdT_d = nc.dram_tensor("pdT", [BL, D, S], F8, kind="ExternalInput")
    pdN_d = nc.dram_tensor("pdN", [BL, S, D], BF16, kind="ExternalInput")
    qwk_d = nc.dram_tensor("qwk", [D, Q1], F8, kind="ExternalInput")
    qwo_d = nc.dram_tensor("qwo", [D, Q1], F8, kind="ExternalInput")
    bq1_d = nc.dram_tensor("bq1", [Q1, 1], F32, kind="ExternalInput")
    bq2_d = nc.dram_tensor("bq2", [Q1, 1], F32, kind="ExternalInput")
    wu2B_d = nc.dram_tensor("wu2B", [Q, D], F32, kind="ExternalInput")
    out_d = nc.dram_tensor("out", [BL, S, D], F32, kind="ExternalOutput")

    # [D, X] -> [128, NJ, X] chunked view for single-descriptor DMA
    def chunked(ap):
        return ap.rearrange("(j p) x -> p j x", p=128)

    with tile.TileContext(nc) as tc:
        with (
            tc.tile_pool(name="const", bufs=1) as cp,
            tc.tile_pool(name="dram", bufs=1, space="DRAM") as dram,
        ):
            ident = cp.tile([128, 128], BF16, tag="ident")
            masks.make_identity(nc, ident[:])

            # per-chunk width padded to 80: DoubleRow LDWEIGHTS requires the
            # inter-subtile step to be a multiple of 16
            QP = 80
            qwk8 = cp.tile([128, NJ, QP], F8, tag="qwk8")
            qwo8 = cp.tile([128, NJ, QP], F8, tag="qwo8")
            bq1 = cp.tile([Q1, 1], F32, tag="bq1")
            bq2 = cp.tile([Q1, 1], F32, tag="bq2")
            wu2B = cp.tile([Q, D], F32, tag="wu2B")

            # critical-path weights first
            nc.sync.dma_start(qwk8[:, :, 0:Q1], chunked(qwk_d[:, :]))
            nc.sync.dma_start(bq1[:], bq1_d[:, :])

            A_acc = cp.tile([Q, D], F32, tag="A_acc")
            A_f32 = cp.tile([Q, D], F32, tag="A_f32")
            A_bf = cp.tile([Q, D], BF16, tag="A_bf")
            # zg rhs: [ones | aw | e_gate] ([Q1, 3]); aw lands after the AR
            awo = cp.tile([Q1, 3], BF16, tag="awo")
            nc.vector.memset(awo[:], 0.0)
            nc.vector.memset(awo[0:Q, 0:1], 1.0)
            nc.vector.memset(awo[Q:Q1, 2:3], 1.0)
            scrA = cp.tile([Q, D], F32, tag="scrA")
            aw = cp.tile([Q, 1], F32, tag="aw")
            ar_in = dram.tile([Q + 1, D + 8], BF16)
            ar_out = dram.tile([Q + 1, D + 8], BF16)
            awz = cp.tile([Q, 8], BF16, tag="awz")
            nc.vector.memset(awz[:], 0.0)
            zrow = cp.tile([1, D + 8], BF16, tag="zrow")
            nc.vector.memset(zrow[:], 0.0)
            nc.gpsimd.dma_start(ar_in[0:Q, D:D + 8], awz[:])
            nc.gpsimd.dma_start(ar_in[Q:Q + 1, Q:D + 8], zrow[:, Q:D + 8])

            # ================= PHASE 1 =================
            with (
                tc.tile_pool(name="p1", bufs=1) as p1,
                tc.tile_pool(name="p1ps", bufs=1, space="PSUM") as p1ps,
            ):
                def load_raw8(b, sb):
                    t = p1.tile([128, NJ, SBLK], F8, tag="raw8",
                                name="raw8", bufs=2)
                    nc.sync.dma_start(
                        t[:], chunked(rawT_d[b])[
                            :, :, sb * SBLK:(sb + 1) * SBLK])
                    return t

                def load_rns(b, sb):
                    t = p1.tile([128, CPB, D], F8, tag="rns",
                                name="rns", bufs=2)
                    nc.sync.dma_start(
                        t[:],
                        rawN_d[b, sb * SBLK:(sb + 1) * SBLK, :].rearrange(
                            "(c p) d -> p c d", p=128))
                    return t

                raw8_cur = load_raw8(0, 0)
                rns_cur = load_rns(0, 0)
                nc.sync.dma_start(qwo8[:, :, 0:Q1], chunked(qwo_d[:, :]))
                nc.sync.dma_start(bq2[:], bq2_d[:, :])
                nc.sync.dma_start(wu2B[:], wu2B_d[:])

                all_p1 = [(b, sb) for b in range(BL) for sb in range(NSB)]
                for b in range(BL):
                    Zp = p1.tile([Q1, NSB], F32, tag="Zp", bufs=2)
                    A_ps = p1ps.tile([Q, D], F32, tag="A_ps", bufs=1)

                    for sb in range(NSB):
                        idx = b * NSB + sb
                        if idx + 1 < len(all_p1):
                            nb, nsb = all_p1[idx + 1]
                            raw8_nxt = load_raw8(nb, nsb)
                            rns_nxt = load_rns(nb, nsb)
                        else:
                            raw8_nxt = rns_nxt = None

                        # scores (+gate row 64) = qwk8.T @ raw8, fp8 DR
                        sc_ps = p1ps.tile([Q1, SBLK], F32, tag="sc_ps",
                                          bufs=2)
                        for j0 in range(NJ // 2):
                            nc.tensor.matmul(
                                sc_ps[:],
                                qwk8[:, 2 * j0:2 * j0 + 2, 0:Q1],
                                raw8_cur[:, 2 * j0:2 * j0 + 2, :],
                                start=(j0 == 0), stop=(j0 == NJ // 2 - 1),
                                perf_mode=DR)
                        # U rows 0..63 = exp(scores); row 64 = exp(-raw.Wi-bi)
                        U = p1.tile([Q1, SBLK], BF16, tag="U", bufs=2)
                        nc.scalar.activation(U[:], sc_ps[:], AF.Exp,
                                             scale=INV_SC, bias=bq1[:],
                                             accum_out=Zp[:, sb:sb + 1])
                        # transpose (65-wide: gate-exp rides in column 64)
                        # per-chunk width padded to 66 so each bf16 slice
                        # stays 4-byte aligned in PSUM
                        ut_ps = p1ps.tile([128, CPB, Q1 + 1], BF16,
                                          tag="ut_ps", bufs=2)
                        for cc in range(CPB):
                            nc.tensor.transpose(
                                ut_ps[:, cc, 0:Q1],
                                U[:, cc * 128:(cc + 1) * 128],
                                ident[:Q1, :Q1])
                        # Gg = sigmoid(raw.Wi+bi) = 1/(1+exp(-)) per chunk
                        Gg = p1.tile([128, CPB], F32, tag="Gg", bufs=2)
                        nc.vector.tensor_scalar_add(
                            Gg[:], ut_ps[:, :, Q:Q + 1].squeeze(), 1.0)
                        nc.vector.reciprocal(Gg[:], Gg[:])
                        uts = p1.tile([128, CPB, Q], F8, tag="uts", bufs=2)
                        for cc in range(CPB):
                            nc.scalar.activation(uts[:, cc, :],
                                                 ut_ps[:, cc, 0:Q], AF.Copy,
                                                 scale=Gg[:, cc:cc + 1])
                        # A += (U*g).T @ rawN, fp8 DR over chunk pairs
                        for c0 in range(CPB // 2):
                            cp2 = sb * (CPB // 2) + c0
                            for h in range(2):
                                hsl = slice(h * 512, (h + 1) * 512)
                                nc.tensor.matmul(
                                    A_ps[:, hsl],
                                    uts[:, 2 * c0:2 * c0 + 2, :],
                                    rns_cur[:, 2 * c0:2 * c0 + 2, hsl],
                                    start=(cp2 == 0),
                                    stop=(cp2 == NSB * CPB // 2 - 1),
                                    perf_mode=DR, skip_group_check=True)
                        raw8_cur, rns_cur = raw8_nxt, rns_nxt

                    # A_acc += A_ps / (16 * B * Z)   (16 = fp8 rawN scale)
                    Z1 = p1.tile([Q, 1], F32, tag="Z1", bufs=2)
                    nc.vector.tensor_reduce(Z1[:], Zp[0:Q, :],
                                            mybir.AxisListType.X, ALU.add)
                    sA = p1.tile([Q, 1], F32, tag="sA", bufs=2)
                    nc.vector.reciprocal(sA[:], Z1[:])
                    nc.vector.tensor_scalar_mul(sA[:], sA[:],
                                                1.0 / (B * RAW_SC))
                    if b == 0:
                        nc.vector.tensor_scalar_mul(A_acc[:], A_ps[:], sA[:])
                    else:
                        nc.vector.scalar_tensor_tensor(
                            A_acc[:], A_ps[:], sA[:], A_acc[:],
                            ALU.mult, ALU.add)

                # aw_local = A_acc . Wu2 rides along in the AllReduce
                nc.vector.scalar_tensor_tensor(
                    scrA[:], A_acc[:], 1.0, wu2B[:],
                    ALU.mult, ALU.mult, accum_out=aw[:])
                nc.gpsimd.dma_start(ar_in[0:Q, 0:D], A_acc[:])
                nc.gpsimd.dma_start(
                    ar_in[Q:Q + 1, 0:Q].rearrange("a b -> b a"), aw[:])

            # ---- AllReduce of partial A across the 8 cores ----
            if stage == "p1":
                nc.sync.dma_start(out_d[0, 0:Q, :], A_acc[:])
            else:
                if stage == "p2":
                    arr = ar_in
                else:
                    nc.gpsimd.collective_compute(
                        "AllReduce", ALU.add,
                        replica_groups=[list(range(NCORES))],
                        ins=[ar_in.opt()], outs=[ar_out.opt()],
                    )
                    arr = ar_out
                if stage == "ar":
                    nc.gpsimd.dma_start(A_f32[:], arr[0:Q, 0:D])
                nc.gpsimd.dma_start(A_bf[:], arr[0:Q, 0:D])
                nc.gpsimd.dma_start(
                    awo[0:Q, 1:2], arr[Q:Q + 1, 0:Q].rearrange("a b -> b a"))

            # ================= PHASE 2 =================
            if stage == "p1":
                pass  # skip phase 2
            else:
              with (
                  tc.tile_pool(name="p2", bufs=1) as p2,
                  tc.tile_pool(name="p2ps", bufs=1, space="PSUM") as p2ps,
              ):
                  NP2 = BL * NSB + 1

                  def part_a(b, sb):
                      ssl = slice(sb * SBLK, (sb + 1) * SBLK)
                      pdt8 = p2.tile([128, NJ, SBLK], F8, tag="pdt8",
                                     name="pdt8", bufs=2)
                      nc.sync.dma_start(pdt8[:], chunked(pdT_d[b])[:, :, ssl])
                      pdn = p2.tile([128, CPB, D], BF16, tag="pdn",
                                    name="pdn", bufs=NP2)
                      nc.sync.dma_start(
                          pdn[:],
                          pdN_d[b, ssl, :].rearrange("(c p) d -> p c d",
                                                     p=128))
                      # s2 (+gate row 64) = qwo8.T @ pdt8, fp8 DR
                      sc2_ps = p2ps.tile([Q1, SBLK], F32, tag="s2_ps",
                                         bufs=2)
                      for j0 in range(NJ // 2):
                          nc.tensor.matmul(
                              sc2_ps[:],
                              qwo8[:, 2 * j0:2 * j0 + 2, 0:Q1],
                              pdt8[:, 2 * j0:2 * j0 + 2, :],
                              start=(j0 == 0), stop=(j0 == NJ // 2 - 1),
                              perf_mode=DR)
                      # U2 rows 0..63 = exp(s2); row 64 = exp(-pd.Wu1-cg)
                      U2 = p2.tile([Q1, SBLK], BF16, tag="U2", bufs=NP2)
                      nc.scalar.activation(U2[:], sc2_ps[:], AF.Exp,
                                           scale=INV_SC, bias=bq2[:])
                      return (U2, pdn, b, sb)

                  def part_b(st):
                      (U2, pdn, b, sb) = st
                      # per chunk: [Z2 | G2 | E1] = U2[q,s].T @ [ones|aw|e64]
                      zg_ps = p2ps.tile([128, CPB, 3], F32, tag="zg",
                                        bufs=2)
                      for cc in range(CPB):
                          nc.tensor.matmul(
                              zg_ps[:, cc, :],
                              U2[:, cc * 128:(cc + 1) * 128], awo[:],
                              start=(cc == 0), stop=(cc == CPB - 1),
                              skip_group_check=True)
                      # SC = sigmoid(G1+G2/Z2+cg)/Z2
                      #    = 1/((1+E1*exp(-G2/Z2))*Z2)
                      rz = p2.tile([128, CPB], F32, tag="rz", bufs=2)
                      nc.vector.reciprocal(rz[:],
                                           zg_ps[:, :, 0:1].squeeze())
                      t4 = p2.tile([128, CPB], F32, tag="t4", bufs=2)
                      nc.vector.tensor_mul(t4[:], zg_ps[:, :, 1:2].squeeze(),
                                           rz[:])
                      e4 = p2.tile([128, CPB], F32, tag="e4", bufs=2)
                      nc.scalar.activation(e4[:], t4[:], AF.Exp, scale=-1.0)
                      nc.vector.tensor_mul(e4[:], e4[:],
                                           zg_ps[:, :, 2:3].squeeze())
                      nc.vector.tensor_scalar_add(e4[:], e4[:], 1.0)
                      nc.vector.reciprocal(e4[:], e4[:])
                      SC = p2.tile([128, CPB], F32, tag="SC", bufs=2)
                      nc.vector.tensor_mul(SC[:], e4[:], rz[:])
                      # A2 + fused residual
                      for cc in range(CPB):
                          c = sb * CPB + cc
                          outt = p2.tile([128, D], F32, tag="outt", bufs=3)
                          for h in range(2):
                              hsl = slice(h * 512, (h + 1) * 512)
                              a2_ps = p2ps.tile([128, 512], F32,
                                                tag="a2_ps", bufs=4)
                              nc.tensor.matmul(
                                  a2_ps[:], U2[0:Q, cc * 128:(cc + 1) * 128],
                                  A_bf[:, hsl], start=True, stop=True)
                              nc.vector.scalar_tensor_tensor(
                                  outt[:, hsl], a2_ps[:],
                                  SC[:, cc:cc + 1],
                                  pdn[:, cc, hsl], ALU.mult, ALU.add)
                          nc.gpsimd.dma_start(
                              out_d[b, c * 128:(c + 1) * 128, :], outt[:])

                  all_sb = [(b, sb) for b in range(BL) for sb in range(NSB)]
                  pending = [part_a(b, sb) for b, sb in all_sb]
                  for st in pending:
                      part_b(st)

            if stage == "ar":
                nc.sync.dma_start(out_d[0, 0:Q, :], A_f32[:])
    nc.compile()
    return nc


def _get_prog():
    if "p" not in _prog_cache:
        _prog_cache["p"] = _build()
    return _prog_cache["p"]


def kernel(raw, post_dec, mask, questions, Wk, bk, Wi, bi, Wo, bo,
           Wu1, bu1, Wu2, bu2, b1, _trace=False):
    raw = np.asarray(raw, dtype=np.float32)
    post_dec = np.asarray(post_dec, dtype=np.float32)
    questions = np.asarray(questions, dtype=np.float32)
    Wk = np.asarray(Wk, dtype=np.float32)
    Wo = np.asarray(Wo, dtype=np.float32)

    bi_v = float(np.asarray(bi).reshape(-1)[0])
    cgate_v = float(np.asarray(bu1).reshape(-1)[0]
                    + np.asarray(bu2).reshape(-1)[0]
                    + np.asarray(b1).reshape(-1)[0])
    nc = _get_prog()

    inv_sqrt_d = np.float32(1.0 / np.sqrt(D))
    inv_sqrt_q = np.float32(1.0 / np.sqrt(Q))

    def to_f8(x):
        return np.clip(x, -240.0, 240.0).astype(F8NP)

    bk_v = np.asarray(bk, np.float32).reshape(D)
    bo_v = np.asarray(bo, np.float32).reshape(D)
    wi_v = np.asarray(Wi, np.float32).reshape(D)
    wu1_v = np.asarray(Wu1, np.float32).reshape(D)

    # host-side tiny projections: scores = raw @ (q@Wk * inv_sqrt_d).T
    qWk = (questions @ Wk) * inv_sqrt_d          # [Q, D]
    qWo = (questions @ Wo) * inv_sqrt_q
    qwk = to_f8(np.concatenate(
        [qWk.T * QW_SC, (-QW_SC) * wi_v[:, None]], axis=1))   # [D, Q1]
    qwo = to_f8(np.concatenate(
        [qWo.T * QW_SC, (-QW_SC) * wu1_v[:, None]], axis=1))
    bq1 = np.concatenate(
        [(questions @ bk_v) * inv_sqrt_d,
         [-bi_v]]).astype(np.float32).reshape(Q1, 1)
    bq2 = np.concatenate(
        [(questions @ bo_v) * inv_sqrt_q,
         [-cgate_v]]).astype(np.float32).reshape(Q1, 1)
    wu2B = np.ascontiguousarray(
        np.broadcast_to(np.asarray(Wu2, np.float32).reshape(1, D), (Q, D)))

    in_maps = []
    for r in range(NCORES):
        bs = slice(r * BL, (r + 1) * BL)
        rawT = to_f8(np.ascontiguousarray(
            raw[bs].transpose(0, 2, 1)) * RAW_SC)
        rawN = to_f8(raw[bs] * RAW_SC)
        pdT = to_f8(np.ascontiguousarray(
            post_dec[bs].transpose(0, 2, 1)) * RAW_SC)
        pdN = np.ascontiguousarray(post_dec[bs]).astype(BF)
        in_maps.append({
            "rawT": rawT, "rawN": rawN, "pdT": pdT, "pdN": pdN,
            "qwk": qwk, "qwo": qwo, "bq1": bq1, "bq2": bq2, "wu2B": wu2B,
        })

    res = run_bass_kernel_spmd(nc, in_maps, core_ids=list(range(NCORES)),
                               trace=_trace)
    out = np.concatenate([res.results[r]["out"] for r in range(NCORES)],
                         axis=0)
    if _trace:
        kernel._last_result = res
    return out
